# revision 9
# baseline (speedup 1.0000x reference)
"""Trainium2 Bass kernel for nn_Gated_GNNML (gated GNN message passing).

Strategy (8 NeuronCores, SPMD):
  - Nodes are permuted and bin-packed into 8 cores x B dst-blocks of 128
    node-slots each, balancing in-degree per block.  Each core owns its
    node range; edges are partitioned by destination owner.
  - The spectral-conv aggregation (segment_sum of h[src] over edges) is
    done per core as: dma_gather of h rows (256B padded bf16 rows) from an
    HBM table, then a one-hot matmul on the tensor engine that scatter-adds
    each 128-edge tile into a [d, 128] PSUM accumulator per dst-block
    (edges pre-sorted by dst-block on host, padded to a uniform tile count
    so the instruction stream is identical across cores).
  - Dense per-node linears run in feature-major (transposed) layout with
    weight-stationary matmuls; relu/sigmoid/mul on ACT/DVE engines.
  - Between GNN blocks, each core transposes its new features to node-major
    rows, AllGathers the compact table, and expands it into padded 256B
    rows for the next round of gathers.
Numerics: bf16 storage / f32 PSUM accumulation (harness gate is 2e-2).
"""

import sys

sys.path.insert(0, "/opt/trn_rl_repo")

import numpy as np
import ml_dtypes

from concourse import bacc, mybir, tile
from concourse.bass_utils import run_bass_kernel_spmd

bf16 = ml_dtypes.bfloat16

NCORES = 8
P = 128  # partitions / node-slots per dst-block


# --------------------------------------------------------------------------
# host-side graph partitioning
# --------------------------------------------------------------------------

def _partition_graph(edge_index, n_nodes):
    """Assign nodes to (core, block, slot) balancing in-degree per block,
    build per-core padded gather-index / dst-slot streams.

    Returns dict with permutation and per-core streams.
    """
    src = np.asarray(edge_index[0], dtype=np.int64)
    dst = np.asarray(edge_index[1], dtype=np.int64)
    E = src.shape[0]

    B = int(np.ceil(n_nodes / (NCORES * P)))  # dst-blocks per core
    OWN = B * P
    NT = NCORES * OWN
    HALF = NT // 2
    assert HALF <= 32768, "gather indices must fit int16"
    NBINS = NCORES * B

    indeg = np.bincount(dst, minlength=n_nodes).astype(np.int64)
    order = np.argsort(-indeg, kind="stable")
    # serpentine round-robin over bins for near-balanced per-bin degree sums
    nfull = (n_nodes // NBINS) * NBINS
    gidx_of_node = np.empty(n_nodes, dtype=np.int64)
    rounds = np.arange(nfull) // NBINS
    pos = np.arange(nfull) % NBINS
    fwd = (rounds % 2) == 0
    binid = np.where(fwd, pos, NBINS - 1 - pos)
    gidx_of_node[order[:nfull]] = binid
    # leftover nodes: bins with smallest current load
    if n_nodes > nfull:
        loads = np.zeros(NBINS, dtype=np.int64)
        np.add.at(loads, gidx_of_node[order[:nfull]], indeg[order[:nfull]])
        rest = order[nfull:]
        pick = np.argsort(loads)[: rest.shape[0]]
        gidx_of_node[rest] = pick
    # slot within bin
    sort_by_bin = np.argsort(gidx_of_node, kind="stable")
    slot = np.empty(n_nodes, dtype=np.int64)
    counts = np.bincount(gidx_of_node, minlength=NBINS)
    assert counts.max() <= P, f"bin overflow {counts.max()}"
    starts = np.zeros(NBINS + 1, dtype=np.int64)
    np.cumsum(counts, out=starts[1:])
    slot[sort_by_bin] = np.arange(n_nodes) - starts[gidx_of_node[sort_by_bin]]

    core = gidx_of_node % NCORES
    blk = gidx_of_node // NCORES
    perm = core * OWN + slot * B + blk  # new node id; table row order
    # SBUF column order within a core: q = blk*P + slot
    qcol = blk * P + slot

    s2 = perm[src]
    d2 = perm[dst]
    e_core = d2 // OWN
    loc = d2 % OWN
    e_p = loc // B
    e_b = loc % B
    e_half = (s2 >= HALF).astype(np.int64)
    e_sidx = (s2 % HALF).astype(np.int64)

    cnt = np.zeros((NCORES, B, 2), dtype=np.int64)
    np.add.at(cnt, (e_core, e_b, e_half), 1)
    T = int(np.ceil(cnt.max() / P))  # tiles per (block, half), uniform
    SLOTS = B * T * P  # per half per core

    # stable order by (core, half, b) then place
    key = ((e_core * 2 + e_half) * B + e_b)
    eorder = np.argsort(key, kind="stable")
    ks = key[eorder]
    # rank within group
    grp_start = np.zeros(NCORES * 2 * B + 1, dtype=np.int64)
    gc = np.bincount(ks, minlength=NCORES * 2 * B)
    np.cumsum(gc, out=grp_start[1:])
    rank = np.arange(E) - grp_start[ks]
    ec = e_core[eorder]
    eh = e_half[eorder]
    eb = e_b[eorder]
    pos_in_stream = eb * (T * P) + rank  # position within (core, half) stream

    gather_idx = np.zeros((NCORES, 2, SLOTS), dtype=np.int16)
    dst_slot = np.full((NCORES, 2, SLOTS), -1.0, dtype=np.float32)
    gather_idx[ec, eh, pos_in_stream] = e_sidx[eorder].astype(np.int16)
    dst_slot[ec, eh, pos_in_stream] = e_p[eorder].astype(np.float32)

    return dict(
        B=B, T=T, OWN=OWN, NT=NT, HALF=HALF,
        perm=perm, core=core, qcol=qcol,
        gather_idx=gather_idx, dst_slot=dst_slot,
    )


def _wrap_idx(ix):
    """int16 stream -> [128, n/16] wrapped layout (i at [i%16, i//16]),
    replicated across the 8 gpsimd core groups."""
    arr = ix.reshape(-1, 16).T  # [16, n/16]
    return np.ascontiguousarray(np.tile(arr, (8, 1)))


# --------------------------------------------------------------------------
# device program
# --------------------------------------------------------------------------

def _build_program(B, T, OWN, NT, HALF, dims, variant="full"):
    """dims: list of per-GNN-block dicts with d_in, names, d_gate etc.
    variant: debug switch - "full", "nocoll" (skip collective; local copy),
    "noexpand" (skip table expand dma), "noexch" (skip whole exchange)."""
    nc = bacc.Bacc("TRN2", target_bir_lowering=False, debug=False,
                   num_devices=NCORES)
    dt = mybir.dt
    SLOTS = B * T * P
    GW = 2 * SLOTS // 16  # gidx free width (int16 cols)

    t_tab1 = nc.dram_tensor("tab1", [NT, 128], dt.bfloat16, kind="ExternalInput")
    t_gidx = nc.dram_tensor("gidx", [128, GW], dt.int16, kind="ExternalInput")
    t_dstc = nc.dram_tensor("dstc", [128, 2 * B * T], dt.float32, kind="ExternalInput")
    t_iota = nc.dram_tensor("iota", [128, 128], dt.bfloat16, kind="ExternalInput")
    t_idnt = nc.dram_tensor("idnt", [128, 128], dt.bfloat16, kind="ExternalInput")
    t_xt = nc.dram_tensor("xt", [64, OWN], dt.bfloat16, kind="ExternalInput")
    t_out = nc.dram_tensor("out", [16, OWN], dt.float32, kind="ExternalOutput")

    wnames = []
    for d in dims:
        wnames += d["linears"]
    t_w = {}
    t_bias = {}
    for name, din, dout in wnames:
        t_w[name] = nc.dram_tensor(f"w_{name}", [din, dout], dt.bfloat16,
                                   kind="ExternalInput")
        t_bias[name] = nc.dram_tensor(f"b_{name}", [dout, 1], dt.float32,
                                      kind="ExternalInput")

    tabs = [t_tab1,
            nc.dram_tensor("tab2", [NT, 128], dt.bfloat16),
            nc.dram_tensor("tab3", [NT, 128], dt.bfloat16)]

    # chunking of dst-blocks for gather granularity
    CHB = 8
    chunks = [(b0, min(CHB, B - b0)) for b0 in range(0, B, CHB)]
    # dense column chunks
    dense_chunks = []
    c0 = 0
    while c0 < OWN:
        w = min(512, OWN - c0)
        dense_chunks.append((c0, w))
        c0 += w

    with tile.TileContext(nc) as tc:
        with tc.tile_pool(name="const", bufs=1) as cpool, \
             tc.tile_pool(name="gpool", bufs=2) as gpool, \
             tc.tile_pool(name="spool", bufs=4) as spool, \
             tc.tile_pool(name="work", bufs=2) as wpool, \
             tc.tile_pool(name="big", bufs=1) as bpool, \
             tc.tile_pool(name="ps_oh", bufs=3, space="PSUM") as ps_oh, \
             tc.tile_pool(name="ps_d", bufs=3, space="PSUM") as ps_d, \
             tc.tile_pool(name="ps_tr", bufs=2, space="PSUM") as ps_tr, \
             tc.tile_pool(name="dram", bufs=1, space="DRAM") as dpool:

            GI = cpool.tile([128, GW], dt.int16)
            DC = cpool.tile([128, 2 * B * T], dt.float32)
            IO = cpool.tile([128, 128], dt.bfloat16)
            ID = cpool.tile([128, 128], dt.bfloat16)
            nc.sync.dma_start(out=GI[:], in_=t_gidx[:])
            nc.sync.dma_start(out=DC[:], in_=t_dstc[:])
            nc.sync.dma_start(out=IO[:], in_=t_iota[:])
            nc.sync.dma_start(out=ID[:], in_=t_idnt[:])

            W = {}
            BIAS = {}
            for name, din, dout in wnames:
                W[name] = cpool.tile([din, dout], dt.bfloat16, tag=f"w_{name}", name=f"w_{name}_t")
                BIAS[name] = cpool.tile([dout, 1], dt.float32, tag=f"b_{name}", name=f"b_{name}_t")
                nc.sync.dma_start(out=W[name][:], in_=t_w[name][:])
                nc.sync.dma_start(out=BIAS[name][:], in_=t_bias[name][:])

            hT = cpool.tile([64, OWN], dt.bfloat16, tag="ht0")
            nc.sync.dma_start(out=hT[:], in_=t_xt[:])

            OUTT = bpool.tile([16, OWN], dt.float32, tag="outT")

            for bi, d in enumerate(dims):
                din = d["d_in"]
                tab = tabs[bi]

                # ---- aggregation: aggT[:, q] = sum_{edges->q} h[src] ----
                aggT = bpool.tile([64, OWN], dt.bfloat16, tag="aggT")
                for (b0, nb) in chunks:
                    G = [None, None]
                    for h in (0, 1):
                        G[h] = gpool.tile([128, CHB * T, 128], dt.bfloat16,
                                          tag=f"g{h}", name=f"g{h}_t")
                        s0 = h * SLOTS + b0 * T * P
                        nidx = nb * T * P
                        nc.gpsimd.dma_gather(
                            G[h][:, 0:nb * T, :],
                            tab[h * HALF:(h + 1) * HALF, :],
                            GI[:, s0 // 16:(s0 + nidx) // 16],
                            nidx, nidx, 128, elem_step=128, single_packet=False)
                    for bb in range(nb):
                        b = b0 + bb
                        ps = ps_oh.tile([64, 128], dt.float32, tag="oh")
                        for h in (0, 1):
                            for t in range(T):
                                col = h * B * T + b * T + t
                                S = spool.tile([128, 128], dt.bfloat16, tag="S")
                                nc.vector.tensor_scalar(
                                    S[:], IO[:], DC[:, col:col + 1], None,
                                    mybir.AluOpType.is_equal)
                                nc.tensor.matmul(
                                    ps[0:din, :], G[h][:, bb * T + t, 0:din],
                                    S[:],
                                    start=(h == 0 and t == 0),
                                    stop=(h == 1 and t == T - 1))
                        nc.vector.tensor_copy(aggT[0:din, b * P:(b + 1) * P],
                                              ps[0:din, :])

                # ---- dense path (feature-major) ----
                f1, f2, f3 = d["f1"], d["f2"], d["f3"]
                cv = d["conv"]
                dmid = din  # block output dim == d_in here
                if d["gate"]:
                    g1, g2 = d["gate"]
                    dg = d["d_gate"]
                    hnext = cpool.tile([dg, OWN], dt.bfloat16, tag=f"ht{bi + 1}")
                else:
                    dg = None
                    hnext = None

                for (c0, wdt) in dense_chunks:
                    sl = slice(c0, c0 + wdt)
                    pA = ps_d.tile([64, 512], dt.float32, tag="pd")
                    nc.tensor.matmul(pA[0:dmid, 0:wdt], W[f1][:],
                                     hT[0:din, sl], start=True, stop=True)
                    r1 = wpool.tile([64, 512], dt.bfloat16, tag="r1")
                    nc.scalar.activation(r1[0:dmid, 0:wdt], pA[0:dmid, 0:wdt],
                                         mybir.ActivationFunctionType.Relu,
                                         bias=BIAS[f1][:])
                    pB = ps_d.tile([64, 512], dt.float32, tag="pd")
                    nc.tensor.matmul(pB[0:dmid, 0:wdt], W[f2][:],
                                     hT[0:din, sl], start=True, stop=True)
                    t2 = wpool.tile([64, 512], dt.bfloat16, tag="t2")
                    nc.vector.tensor_scalar(t2[0:dmid, 0:wdt], pB[0:dmid, 0:wdt],
                                            BIAS[f2][:], None,
                                            mybir.AluOpType.add)
                    pC = ps_d.tile([64, 512], dt.float32, tag="pd")
                    nc.tensor.matmul(pC[0:dmid, 0:wdt], W[f3][:],
                                     hT[0:din, sl], start=True, stop=True)
                    t3 = wpool.tile([64, 512], dt.bfloat16, tag="t3")
                    nc.vector.tensor_scalar(t3[0:dmid, 0:wdt], pC[0:dmid, 0:wdt],
                                            BIAS[f3][:], None,
                                            mybir.AluOpType.add)
                    r3 = wpool.tile([64, 512], dt.bfloat16, tag="r3")
                    nc.vector.tensor_tensor(r3[0:dmid, 0:wdt], t2[0:dmid, 0:wdt],
                                            t3[0:dmid, 0:wdt],
                                            mybir.AluOpType.mult)
                    nc.vector.tensor_scalar_max(r3[0:dmid, 0:wdt],
                                                r3[0:dmid, 0:wdt], 0.0)
                    pD = ps_d.tile([64, 512], dt.float32, tag="pd")
                    nc.tensor.matmul(pD[0:dmid, 0:wdt], W[cv][:],
                                     aggT[0:din, sl], start=True, stop=True)
                    r2 = wpool.tile([64, 512], dt.bfloat16, tag="r2")
                    nc.scalar.activation(r2[0:dmid, 0:wdt], pD[0:dmid, 0:wdt],
                                         mybir.ActivationFunctionType.Relu,
                                         bias=BIAS[cv][:])
                    hs = wpool.tile([64, 512], dt.bfloat16, tag="hs")
                    nc.vector.tensor_tensor(hs[0:dmid, 0:wdt], r1[0:dmid, 0:wdt],
                                            r2[0:dmid, 0:wdt],
                                            mybir.AluOpType.add)
                    nc.vector.tensor_tensor(hs[0:dmid, 0:wdt], hs[0:dmid, 0:wdt],
                                            r3[0:dmid, 0:wdt],
                                            mybir.AluOpType.add)
                    if d["gate"]:
                        pE = ps_d.tile([64, 512], dt.float32, tag="pd")
                        nc.tensor.matmul(pE[0:dg, 0:wdt], W[g1][:],
                                         hs[0:dmid, 0:wdt], start=True, stop=True)
                        sg = wpool.tile([64, 512], dt.bfloat16, tag="sg")
                        nc.scalar.activation(sg[0:dg, 0:wdt], pE[0:dg, 0:wdt],
                                             mybir.ActivationFunctionType.Sigmoid,
                                             bias=BIAS[g1][:])
                        pF = ps_d.tile([64, 512], dt.float32, tag="pd")
                        nc.tensor.matmul(pF[0:dg, 0:wdt], W[g2][:],
                                         hs[0:dmid, 0:wdt], start=True, stop=True)
                        tg = wpool.tile([64, 512], dt.bfloat16, tag="tg")
                        nc.vector.tensor_scalar(tg[0:dg, 0:wdt], pF[0:dg, 0:wdt],
                                                BIAS[g2][:], None,
                                                mybir.AluOpType.add)
                        nc.vector.tensor_tensor(hnext[0:dg, sl], sg[0:dg, 0:wdt],
                                                tg[0:dg, 0:wdt],
                                                mybir.AluOpType.mult)
                    else:
                        fcl = d["final"]
                        pG = ps_d.tile([64, 512], dt.float32, tag="pd")
                        nc.tensor.matmul(pG[0:16, 0:wdt], W[fcl][:],
                                         hs[0:dmid, 0:wdt], start=True, stop=True)
                        nc.vector.tensor_scalar(OUTT[0:16, sl], pG[0:16, 0:wdt],
                                                BIAS[fcl][:], None,
                                                mybir.AluOpType.add)

                # ---- table exchange for next block ----
                if bi < 2:
                    if variant != "noexch":
                        HNM = bpool.tile([128, B * dg], dt.bfloat16, tag="hnm")
                        for b in range(B):
                            pt = ps_tr.tile([128, 64], dt.bfloat16, tag="tr")
                            nc.tensor.transpose(pt[:, 0:dg],
                                                hnext[0:dg, b * P:(b + 1) * P],
                                                ID[0:dg, 0:dg])
                            nc.vector.tensor_copy(HNM[:, b * dg:(b + 1) * dg],
                                                  pt[:, 0:dg])
                        ag_in = dpool.tile([128, B * dg], dt.bfloat16,
                                           tag=f"agin{bi}")
                        ag_out = dpool.tile([NT, dg], dt.bfloat16, tag=f"agout{bi}")
                        nc.sync.dma_start(out=ag_in[:], in_=HNM[:])
                        if variant == "nocoll":
                            for cc in range(NCORES):
                                nc.sync.dma_start(
                                    out=ag_out[cc * OWN:(cc + 1) * OWN, :],
                                    in_=ag_in.opt())
                        else:
                            nc.gpsimd.collective_compute(
                                "AllGather", mybir.AluOpType.bypass,
                                replica_groups=[list(range(NCORES))],
                                ins=[ag_in.opt()], outs=[ag_out.opt()])
                        if variant != "noexpand":
                            nc.sync.dma_start(out=tabs[bi + 1][:, 0:dg],
                                              in_=ag_out[:])
                    hT = hnext

            nc.sync.dma_start(out=t_out[:], in_=OUTT[:])

    nc.compile()
    return nc


_CACHE = {}


def _dims():
    return [
        dict(d_in=64, f1="fc11", f2="fc12", f3="fc13", conv="conv11",
             gate=("gate1_fc1", "gate1_fc2"), d_gate=32,
             linears=[("fc11", 64, 64), ("fc12", 64, 64), ("fc13", 64, 64),
                      ("conv11", 64, 64), ("gate1_fc1", 64, 32),
                      ("gate1_fc2", 64, 32)]),
        dict(d_in=32, f1="fc21", f2="fc22", f3="fc23", conv="conv21",
             gate=("gate2_fc1", "gate2_fc2"), d_gate=16,
             linears=[("fc21", 32, 32), ("fc22", 32, 32), ("fc23", 32, 32),
                      ("conv21", 32, 32), ("gate2_fc1", 32, 16),
                      ("gate2_fc2", 32, 16)]),
        dict(d_in=16, f1="fc31", f2="fc32", f3="fc33", conv="conv31",
             gate=None, final="fc2",
             linears=[("fc31", 16, 16), ("fc32", 16, 16), ("fc33", 16, 16),
                      ("conv31", 16, 16), ("fc2", 16, 16)]),
    ]


def _make_in_maps(x, params, part, dims):
    N = x.shape[0]
    d64 = x.shape[1]
    B, OWN, NT = part["B"], part["OWN"], part["NT"]
    perm, core, qcol = part["perm"], part["core"], part["qcol"]

    tab1 = np.zeros((NT, 128), dtype=bf16)
    tab1[perm, 0:d64] = x.astype(bf16)

    iota = np.tile(np.arange(128, dtype=np.float32).astype(bf16)[None, :],
                   (128, 1))
    ident = np.eye(128, dtype=np.float32).astype(bf16)

    shared = {"tab1": tab1, "iota": np.ascontiguousarray(iota),
              "idnt": np.ascontiguousarray(ident)}
    for d in dims:
        for name, din, dout in d["linears"]:
            shared[f"w_{name}"] = params[name + "_w"].astype(bf16)
            shared[f"b_{name}"] = np.ascontiguousarray(
                params[name + "_b"].reshape(dout, 1))

    in_maps = []
    for c in range(NCORES):
        m = dict(shared)
        gi = np.concatenate([part["gather_idx"][c, 0], part["gather_idx"][c, 1]])
        m["gidx"] = _wrap_idx(gi)
        # dst-slot columns: [128, 2*B*T]; tile t's 128 values wrapped by edge slot
        dsl = np.concatenate([part["dst_slot"][c, 0], part["dst_slot"][c, 1]])
        m["dstc"] = np.ascontiguousarray(dsl.reshape(-1, 128).T)
        # xT for owned nodes, column order q = blk*P + slot
        own_nodes = np.where(core == c)[0]
        xt = np.zeros((64, OWN), dtype=bf16)
        xt[0:d64, qcol[own_nodes]] = x[own_nodes].T.astype(bf16)
        m["xt"] = xt
        in_maps.append(m)
    return in_maps


def kernel(x, params, edge_index):
    x = np.asarray(x, dtype=np.float32)
    edge_index = np.asarray(edge_index)
    params = {k: np.asarray(v, dtype=np.float32) for k, v in params.items()}
    N = x.shape[0]

    part = _partition_graph(edge_index, N)
    B, T, OWN, NT, HALF = part["B"], part["T"], part["OWN"], part["NT"], part["HALF"]
    core, qcol = part["core"], part["qcol"]
    dims = _dims()

    key = (B, T, OWN, N, x.shape[1])
    if key not in _CACHE:
        _CACHE[key] = _build_program(B, T, OWN, NT, HALF, dims)
    nc = _CACHE[key]

    in_maps = _make_in_maps(x, params, part, dims)
    results = run_bass_kernel_spmd(nc, in_maps, list(range(NCORES))).results

    out = np.empty((N, 16), dtype=np.float32)
    for c in range(NCORES):
        own_nodes = np.where(core == c)[0]
        out[own_nodes] = results[c]["out"].T[qcol[own_nodes]]
    return out


# revision 10
# speedup vs baseline: 1.2991x; 1.2991x over previous
"""Trainium2 Bass kernel for nn_Gated_GNNML (gated GNN message passing).

Strategy (8 NeuronCores, SPMD):
  - Nodes are permuted and bin-packed into 8 cores x B dst-blocks of 128
    node-slots each, balancing in-degree per block.  Each core owns its
    node range; edges are partitioned by destination owner.
  - The spectral-conv aggregation (segment_sum of h[src] over edges) is
    done per core as: dma_gather of h rows (256B padded bf16 rows) from an
    HBM table, then a one-hot matmul on the tensor engine that scatter-adds
    each 128-edge tile into a [d, 128] PSUM accumulator per dst-block
    (edges pre-sorted by dst-block on host, padded to a uniform tile count
    so the instruction stream is identical across cores).
  - Dense per-node linears run in feature-major (transposed) layout with
    weight-stationary matmuls; relu/sigmoid/mul on ACT/DVE engines.
  - Between GNN blocks, each core transposes its new features to node-major
    rows, AllGathers the compact table, and expands it into padded 256B
    rows for the next round of gathers.
Numerics: bf16 storage / f32 PSUM accumulation (harness gate is 2e-2).
"""

import sys

sys.path.insert(0, "/opt/trn_rl_repo")

import numpy as np
import ml_dtypes

from concourse import bacc, mybir, tile
from concourse.bass_utils import run_bass_kernel_spmd

bf16 = ml_dtypes.bfloat16

NCORES = 8
P = 128  # partitions / node-slots per dst-block


# --------------------------------------------------------------------------
# host-side graph partitioning
# --------------------------------------------------------------------------

def _partition_graph(edge_index, n_nodes):
    """Assign nodes to (core, block, slot) balancing in-degree per block,
    build per-core padded gather-index / dst-slot streams.

    Returns dict with permutation and per-core streams.
    """
    src = np.asarray(edge_index[0], dtype=np.int64)
    dst = np.asarray(edge_index[1], dtype=np.int64)
    E = src.shape[0]

    B = int(np.ceil(n_nodes / (NCORES * P)))  # dst-blocks per core
    OWN = B * P
    NT = NCORES * OWN
    HALF = NT // 2
    assert HALF <= 32768, "gather indices must fit int16"
    NBINS = NCORES * B

    indeg = np.bincount(dst, minlength=n_nodes).astype(np.int64)
    order = np.argsort(-indeg, kind="stable")
    # serpentine round-robin over bins for near-balanced per-bin degree sums
    nfull = (n_nodes // NBINS) * NBINS
    gidx_of_node = np.empty(n_nodes, dtype=np.int64)
    rounds = np.arange(nfull) // NBINS
    pos = np.arange(nfull) % NBINS
    fwd = (rounds % 2) == 0
    binid = np.where(fwd, pos, NBINS - 1 - pos)
    gidx_of_node[order[:nfull]] = binid
    # leftover nodes: bins with smallest current load
    if n_nodes > nfull:
        loads = np.zeros(NBINS, dtype=np.int64)
        np.add.at(loads, gidx_of_node[order[:nfull]], indeg[order[:nfull]])
        rest = order[nfull:]
        pick = np.argsort(loads)[: rest.shape[0]]
        gidx_of_node[rest] = pick
    # slot within bin
    sort_by_bin = np.argsort(gidx_of_node, kind="stable")
    slot = np.empty(n_nodes, dtype=np.int64)
    counts = np.bincount(gidx_of_node, minlength=NBINS)
    assert counts.max() <= P, f"bin overflow {counts.max()}"
    starts = np.zeros(NBINS + 1, dtype=np.int64)
    np.cumsum(counts, out=starts[1:])
    slot[sort_by_bin] = np.arange(n_nodes) - starts[gidx_of_node[sort_by_bin]]

    core = gidx_of_node % NCORES
    blk = gidx_of_node // NCORES
    perm = core * OWN + slot * B + blk  # new node id; table row order
    # SBUF column order within a core: q = blk*P + slot
    qcol = blk * P + slot

    s2 = perm[src]
    d2 = perm[dst]
    e_core = d2 // OWN
    loc = d2 % OWN
    e_p = loc // B
    e_b = loc % B
    e_half = (s2 >= HALF).astype(np.int64)
    e_sidx = (s2 % HALF).astype(np.int64)

    cnt = np.zeros((NCORES, B, 2), dtype=np.int64)
    np.add.at(cnt, (e_core, e_b, e_half), 1)
    T = int(np.ceil(cnt.max() / P))  # tiles per (block, half), uniform
    SLOTS = B * T * P  # per half per core

    # stable order by (core, half, b) then place
    key = ((e_core * 2 + e_half) * B + e_b)
    eorder = np.argsort(key, kind="stable")
    ks = key[eorder]
    # rank within group
    grp_start = np.zeros(NCORES * 2 * B + 1, dtype=np.int64)
    gc = np.bincount(ks, minlength=NCORES * 2 * B)
    np.cumsum(gc, out=grp_start[1:])
    rank = np.arange(E) - grp_start[ks]
    ec = e_core[eorder]
    eh = e_half[eorder]
    eb = e_b[eorder]
    pos_in_stream = eb * (T * P) + rank  # position within (core, half) stream

    gather_idx = np.zeros((NCORES, 2, SLOTS), dtype=np.int16)
    dst_slot = np.full((NCORES, 2, SLOTS), -1.0, dtype=np.float32)
    gather_idx[ec, eh, pos_in_stream] = e_sidx[eorder].astype(np.int16)
    dst_slot[ec, eh, pos_in_stream] = e_p[eorder].astype(np.float32)

    return dict(
        B=B, T=T, OWN=OWN, NT=NT, HALF=HALF,
        perm=perm, core=core, qcol=qcol,
        gather_idx=gather_idx, dst_slot=dst_slot,
    )


def _wrap_idx(ix):
    """int16 stream -> [128, n/16] wrapped layout (i at [i%16, i//16]),
    replicated across the 8 gpsimd core groups."""
    arr = ix.reshape(-1, 16).T  # [16, n/16]
    return np.ascontiguousarray(np.tile(arr, (8, 1)))


# --------------------------------------------------------------------------
# device program
# --------------------------------------------------------------------------

def _build_program(B, T, OWN, NT, HALF, dims, variant="full"):
    """dims: list of per-GNN-block dicts with d_in, names, d_gate etc.
    variant: debug switch - "full", "nocoll" (skip collective; local copy),
    "noexpand" (skip table expand dma), "noexch" (skip whole exchange)."""
    nc = bacc.Bacc("TRN2", target_bir_lowering=False, debug=False,
                   num_devices=NCORES)
    dt = mybir.dt
    SLOTS = B * T * P
    GW = 2 * SLOTS // 16  # gidx free width (int16 cols)

    t_tab1 = nc.dram_tensor("tab1", [NT, 128], dt.bfloat16, kind="ExternalInput")
    t_gidx = nc.dram_tensor("gidx", [128, GW], dt.int16, kind="ExternalInput")
    t_dstc = nc.dram_tensor("dstc", [128, 2 * B * T], dt.float32, kind="ExternalInput")
    t_iota = nc.dram_tensor("iota", [128, 128], dt.bfloat16, kind="ExternalInput")
    t_idnt = nc.dram_tensor("idnt", [128, 128], dt.bfloat16, kind="ExternalInput")
    t_xt = nc.dram_tensor("xt", [64, OWN], dt.bfloat16, kind="ExternalInput")
    t_out = nc.dram_tensor("out", [16, OWN], dt.float32, kind="ExternalOutput")

    wnames = []
    for d in dims:
        wnames += d["linears"]
    t_w = {}
    t_bias = {}
    for name, din, dout in wnames:
        t_w[name] = nc.dram_tensor(f"w_{name}", [din, dout], dt.bfloat16,
                                   kind="ExternalInput")
        t_bias[name] = nc.dram_tensor(f"b_{name}", [dout, 1], dt.float32,
                                      kind="ExternalInput")

    tabs = [t_tab1,
            nc.dram_tensor("tab2", [NT, 128], dt.bfloat16),
            nc.dram_tensor("tab3", [NT, 128], dt.bfloat16)]

    # chunking of dst-blocks for gather granularity
    CHB = 8
    chunks = [(b0, min(CHB, B - b0)) for b0 in range(0, B, CHB)]
    # dense column chunks
    dense_chunks = []
    c0 = 0
    while c0 < OWN:
        w = min(512, OWN - c0)
        dense_chunks.append((c0, w))
        c0 += w

    with tile.TileContext(nc) as tc:
        with tc.tile_pool(name="const", bufs=1) as cpool, \
             tc.tile_pool(name="gpool", bufs=2) as gpool, \
             tc.tile_pool(name="spool", bufs=4) as spool, \
             tc.tile_pool(name="work", bufs=2) as wpool, \
             tc.tile_pool(name="big", bufs=1) as bpool, \
             tc.tile_pool(name="ps_oh", bufs=3, space="PSUM") as ps_oh, \
             tc.tile_pool(name="ps_d", bufs=3, space="PSUM") as ps_d, \
             tc.tile_pool(name="ps_tr", bufs=2, space="PSUM") as ps_tr, \
             tc.tile_pool(name="dram", bufs=1, space="DRAM") as dpool:

            GI = cpool.tile([128, GW], dt.int16)
            DC = cpool.tile([128, 2 * B * T], dt.float32)
            IO = cpool.tile([128, 128], dt.bfloat16)
            ID = cpool.tile([128, 128], dt.bfloat16)
            nc.sync.dma_start(out=GI[:], in_=t_gidx[:])
            nc.sync.dma_start(out=DC[:], in_=t_dstc[:])
            nc.sync.dma_start(out=IO[:], in_=t_iota[:])
            nc.sync.dma_start(out=ID[:], in_=t_idnt[:])

            W = {}
            BIAS = {}
            for name, din, dout in wnames:
                W[name] = cpool.tile([din, dout], dt.bfloat16, tag=f"w_{name}", name=f"w_{name}_t")
                BIAS[name] = cpool.tile([dout, 1], dt.float32, tag=f"b_{name}", name=f"b_{name}_t")
                nc.sync.dma_start(out=W[name][:], in_=t_w[name][:])
                nc.sync.dma_start(out=BIAS[name][:], in_=t_bias[name][:])

            hT = cpool.tile([64, OWN], dt.bfloat16, tag="ht0")
            nc.sync.dma_start(out=hT[:], in_=t_xt[:])

            OUTT = bpool.tile([16, OWN], dt.float32, tag="outT")

            for bi, d in enumerate(dims):
                din = d["d_in"]
                tab = tabs[bi]

                # ---- aggregation: aggT[:, q] = sum_{edges->q} h[src] ----
                aggT = bpool.tile([64, OWN], dt.bfloat16, tag="aggT")
                for (b0, nb) in chunks:
                    G = [None, None]
                    for h in (0, 1):
                        G[h] = gpool.tile([128, CHB * T, 128], dt.bfloat16,
                                          tag=f"g{h}", name=f"g{h}_t")
                        total_tiles = nb * T
                        # <=1024 idxs per call (HW cap for single-packet
                        # SWDGE gathers); single_packet keeps desc-gen fast
                        for k0 in range(0, total_tiles, 8):
                            kt = min(8, total_tiles - k0)
                            nidx = kt * P
                            s0 = h * SLOTS + (b0 * T + k0) * P
                            nc.gpsimd.dma_gather(
                                G[h][:, k0:k0 + kt, :],
                                tab[h * HALF:(h + 1) * HALF, :],
                                GI[:, s0 // 16:(s0 + nidx) // 16],
                                nidx, nidx, 128, elem_step=128)
                    for bb in range(nb):
                        b = b0 + bb
                        ps = ps_oh.tile([64, 128], dt.float32, tag="oh")
                        for h in (0, 1):
                            for t in range(T):
                                col = h * B * T + b * T + t
                                S = spool.tile([128, 128], dt.bfloat16, tag="S")
                                nc.vector.tensor_scalar(
                                    S[:], IO[:], DC[:, col:col + 1], None,
                                    mybir.AluOpType.is_equal)
                                nc.tensor.matmul(
                                    ps[0:din, :], G[h][:, bb * T + t, 0:din],
                                    S[:],
                                    start=(h == 0 and t == 0),
                                    stop=(h == 1 and t == T - 1))
                        nc.vector.tensor_copy(aggT[0:din, b * P:(b + 1) * P],
                                              ps[0:din, :])

                # ---- dense path (feature-major) ----
                f1, f2, f3 = d["f1"], d["f2"], d["f3"]
                cv = d["conv"]
                dmid = din  # block output dim == d_in here
                if d["gate"]:
                    g1, g2 = d["gate"]
                    dg = d["d_gate"]
                    hnext = cpool.tile([dg, OWN], dt.bfloat16, tag=f"ht{bi + 1}")
                else:
                    dg = None
                    hnext = None

                for (c0, wdt) in dense_chunks:
                    sl = slice(c0, c0 + wdt)
                    pA = ps_d.tile([64, 512], dt.float32, tag="pd")
                    nc.tensor.matmul(pA[0:dmid, 0:wdt], W[f1][:],
                                     hT[0:din, sl], start=True, stop=True)
                    r1 = wpool.tile([64, 512], dt.bfloat16, tag="r1")
                    nc.scalar.activation(r1[0:dmid, 0:wdt], pA[0:dmid, 0:wdt],
                                         mybir.ActivationFunctionType.Relu,
                                         bias=BIAS[f1][:])
                    pB = ps_d.tile([64, 512], dt.float32, tag="pd")
                    nc.tensor.matmul(pB[0:dmid, 0:wdt], W[f2][:],
                                     hT[0:din, sl], start=True, stop=True)
                    t2 = wpool.tile([64, 512], dt.bfloat16, tag="t2")
                    nc.vector.tensor_scalar(t2[0:dmid, 0:wdt], pB[0:dmid, 0:wdt],
                                            BIAS[f2][:], None,
                                            mybir.AluOpType.add)
                    pC = ps_d.tile([64, 512], dt.float32, tag="pd")
                    nc.tensor.matmul(pC[0:dmid, 0:wdt], W[f3][:],
                                     hT[0:din, sl], start=True, stop=True)
                    t3 = wpool.tile([64, 512], dt.bfloat16, tag="t3")
                    nc.vector.tensor_scalar(t3[0:dmid, 0:wdt], pC[0:dmid, 0:wdt],
                                            BIAS[f3][:], None,
                                            mybir.AluOpType.add)
                    r3 = wpool.tile([64, 512], dt.bfloat16, tag="r3")
                    nc.vector.tensor_tensor(r3[0:dmid, 0:wdt], t2[0:dmid, 0:wdt],
                                            t3[0:dmid, 0:wdt],
                                            mybir.AluOpType.mult)
                    nc.vector.tensor_scalar_max(r3[0:dmid, 0:wdt],
                                                r3[0:dmid, 0:wdt], 0.0)
                    pD = ps_d.tile([64, 512], dt.float32, tag="pd")
                    nc.tensor.matmul(pD[0:dmid, 0:wdt], W[cv][:],
                                     aggT[0:din, sl], start=True, stop=True)
                    r2 = wpool.tile([64, 512], dt.bfloat16, tag="r2")
                    nc.scalar.activation(r2[0:dmid, 0:wdt], pD[0:dmid, 0:wdt],
                                         mybir.ActivationFunctionType.Relu,
                                         bias=BIAS[cv][:])
                    hs = wpool.tile([64, 512], dt.bfloat16, tag="hs")
                    nc.vector.tensor_tensor(hs[0:dmid, 0:wdt], r1[0:dmid, 0:wdt],
                                            r2[0:dmid, 0:wdt],
                                            mybir.AluOpType.add)
                    nc.vector.tensor_tensor(hs[0:dmid, 0:wdt], hs[0:dmid, 0:wdt],
                                            r3[0:dmid, 0:wdt],
                                            mybir.AluOpType.add)
                    if d["gate"]:
                        pE = ps_d.tile([64, 512], dt.float32, tag="pd")
                        nc.tensor.matmul(pE[0:dg, 0:wdt], W[g1][:],
                                         hs[0:dmid, 0:wdt], start=True, stop=True)
                        sg = wpool.tile([64, 512], dt.bfloat16, tag="sg")
                        nc.scalar.activation(sg[0:dg, 0:wdt], pE[0:dg, 0:wdt],
                                             mybir.ActivationFunctionType.Sigmoid,
                                             bias=BIAS[g1][:])
                        pF = ps_d.tile([64, 512], dt.float32, tag="pd")
                        nc.tensor.matmul(pF[0:dg, 0:wdt], W[g2][:],
                                         hs[0:dmid, 0:wdt], start=True, stop=True)
                        tg = wpool.tile([64, 512], dt.bfloat16, tag="tg")
                        nc.vector.tensor_scalar(tg[0:dg, 0:wdt], pF[0:dg, 0:wdt],
                                                BIAS[g2][:], None,
                                                mybir.AluOpType.add)
                        nc.vector.tensor_tensor(hnext[0:dg, sl], sg[0:dg, 0:wdt],
                                                tg[0:dg, 0:wdt],
                                                mybir.AluOpType.mult)
                    else:
                        fcl = d["final"]
                        pG = ps_d.tile([64, 512], dt.float32, tag="pd")
                        nc.tensor.matmul(pG[0:16, 0:wdt], W[fcl][:],
                                         hs[0:dmid, 0:wdt], start=True, stop=True)
                        nc.vector.tensor_scalar(OUTT[0:16, sl], pG[0:16, 0:wdt],
                                                BIAS[fcl][:], None,
                                                mybir.AluOpType.add)

                # ---- table exchange for next block ----
                if bi < 2:
                    if variant != "noexch":
                        HNM = bpool.tile([128, B * dg], dt.bfloat16, tag="hnm")
                        for b in range(B):
                            pt = ps_tr.tile([128, 64], dt.bfloat16, tag="tr")
                            nc.tensor.transpose(pt[:, 0:dg],
                                                hnext[0:dg, b * P:(b + 1) * P],
                                                ID[0:dg, 0:dg])
                            nc.vector.tensor_copy(HNM[:, b * dg:(b + 1) * dg],
                                                  pt[:, 0:dg])
                        ag_in = dpool.tile([128, B * dg], dt.bfloat16,
                                           tag=f"agin{bi}")
                        ag_out = dpool.tile([NT, dg], dt.bfloat16, tag=f"agout{bi}")
                        nc.sync.dma_start(out=ag_in[:], in_=HNM[:])
                        if variant == "nocoll":
                            for cc in range(NCORES):
                                nc.sync.dma_start(
                                    out=ag_out[cc * OWN:(cc + 1) * OWN, :],
                                    in_=ag_in.opt())
                        else:
                            nc.gpsimd.collective_compute(
                                "AllGather", mybir.AluOpType.bypass,
                                replica_groups=[list(range(NCORES))],
                                ins=[ag_in.opt()], outs=[ag_out.opt()])
                        if variant != "noexpand":
                            nc.sync.dma_start(out=tabs[bi + 1][:, 0:dg],
                                              in_=ag_out[:])
                    hT = hnext

            nc.sync.dma_start(out=t_out[:], in_=OUTT[:])

    nc.compile()
    return nc


_CACHE = {}


def _dims():
    return [
        dict(d_in=64, f1="fc11", f2="fc12", f3="fc13", conv="conv11",
             gate=("gate1_fc1", "gate1_fc2"), d_gate=32,
             linears=[("fc11", 64, 64), ("fc12", 64, 64), ("fc13", 64, 64),
                      ("conv11", 64, 64), ("gate1_fc1", 64, 32),
                      ("gate1_fc2", 64, 32)]),
        dict(d_in=32, f1="fc21", f2="fc22", f3="fc23", conv="conv21",
             gate=("gate2_fc1", "gate2_fc2"), d_gate=16,
             linears=[("fc21", 32, 32), ("fc22", 32, 32), ("fc23", 32, 32),
                      ("conv21", 32, 32), ("gate2_fc1", 32, 16),
                      ("gate2_fc2", 32, 16)]),
        dict(d_in=16, f1="fc31", f2="fc32", f3="fc33", conv="conv31",
             gate=None, final="fc2",
             linears=[("fc31", 16, 16), ("fc32", 16, 16), ("fc33", 16, 16),
                      ("conv31", 16, 16), ("fc2", 16, 16)]),
    ]


def _make_in_maps(x, params, part, dims):
    N = x.shape[0]
    d64 = x.shape[1]
    B, OWN, NT = part["B"], part["OWN"], part["NT"]
    perm, core, qcol = part["perm"], part["core"], part["qcol"]

    tab1 = np.zeros((NT, 128), dtype=bf16)
    tab1[perm, 0:d64] = x.astype(bf16)

    iota = np.tile(np.arange(128, dtype=np.float32).astype(bf16)[None, :],
                   (128, 1))
    ident = np.eye(128, dtype=np.float32).astype(bf16)

    shared = {"tab1": tab1, "iota": np.ascontiguousarray(iota),
              "idnt": np.ascontiguousarray(ident)}
    for d in dims:
        for name, din, dout in d["linears"]:
            shared[f"w_{name}"] = params[name + "_w"].astype(bf16)
            shared[f"b_{name}"] = np.ascontiguousarray(
                params[name + "_b"].reshape(dout, 1))

    in_maps = []
    for c in range(NCORES):
        m = dict(shared)
        gi = np.concatenate([part["gather_idx"][c, 0], part["gather_idx"][c, 1]])
        m["gidx"] = _wrap_idx(gi)
        # dst-slot columns: [128, 2*B*T]; tile t's 128 values wrapped by edge slot
        dsl = np.concatenate([part["dst_slot"][c, 0], part["dst_slot"][c, 1]])
        m["dstc"] = np.ascontiguousarray(dsl.reshape(-1, 128).T)
        # xT for owned nodes, column order q = blk*P + slot
        own_nodes = np.where(core == c)[0]
        xt = np.zeros((64, OWN), dtype=bf16)
        xt[0:d64, qcol[own_nodes]] = x[own_nodes].T.astype(bf16)
        m["xt"] = xt
        in_maps.append(m)
    return in_maps


def kernel(x, params, edge_index):
    x = np.asarray(x, dtype=np.float32)
    edge_index = np.asarray(edge_index)
    params = {k: np.asarray(v, dtype=np.float32) for k, v in params.items()}
    N = x.shape[0]

    part = _partition_graph(edge_index, N)
    B, T, OWN, NT, HALF = part["B"], part["T"], part["OWN"], part["NT"], part["HALF"]
    core, qcol = part["core"], part["qcol"]
    dims = _dims()

    key = (B, T, OWN, N, x.shape[1])
    if key not in _CACHE:
        _CACHE[key] = _build_program(B, T, OWN, NT, HALF, dims)
    nc = _CACHE[key]

    in_maps = _make_in_maps(x, params, part, dims)
    results = run_bass_kernel_spmd(nc, in_maps, list(range(NCORES))).results

    out = np.empty((N, 16), dtype=np.float32)
    for c in range(NCORES):
        own_nodes = np.where(core == c)[0]
        out[own_nodes] = results[c]["out"].T[qcol[own_nodes]]
    return out
